# revision 7
# baseline (speedup 1.0000x reference)
"""Trainium2 Bass kernel for nn_KVAttnDecoderRNN (GRU decoder step + dot attention
+ KB embedding branch + vocab projection), tensor-parallel over 8 NeuronCores.

Sharding:
  - GRU gates sharded over hidden (64 rows/core), h1 AllGather (tiny).
  - Attention sharded over L=512 (64 l/core); softmax over batch is local per l;
    context partials AllReduce (20KB).
  - Out projection sharded over vocab (4000 rows/core), host concat.
  - KB embedding gather sharded over the pad dim (64/core) via dma_gather.
All per-core variation is carried by input DATA (sliced/transposed on host) so a
single SPMD program runs on all 8 cores.

Perf notes (HWDGE transfers serialize per ring; ACT LUT reloads on function
switch): all small inputs ship as ONE packed blob DMA laid out in final SBUF
form (ones vector, biases, int16 gather indices bitcast into fp32 columns);
the 8MB w_out slab is one DMA with 16KB contiguous runs; E/w_concat and
collective staging ride the scalar HWDGE ring; psum->sbuf copies stay on DVE;
the kb gather is split around the AllGather trigger on the gpsimd queue.
"""

import math
import os
import sys

import numpy as np

for _p in ("/opt/trn_rl_repo", "/root/.axon_site/_ro/trn_rl_repo"):
    if _p not in sys.path:
        sys.path.append(_p)

B = 10
H = 512
VOCAB = 32000
KB = 431
KB_PAD = 1523
NCORES = 8
VS = VOCAB // NCORES  # 4000 vocab rows per core
LS = H // NCORES      # 64 attention positions per core
GS = H // NCORES      # 64 GRU gate rows per core
FLAT = KB * H         # 220672 flat e2 elements per batch
SLAB = FLAT // NCORES  # 27584 flat elements per core per batch
NJ = 64               # padded kb rows per (core, batch)
NIDX = 3 * B * NJ     # 1920 gather indices per core
NI1 = B * NJ          # first gather: t=0 block (640)
NI2 = 2 * B * NJ      # second gather: t=1,2 blocks (1280)

# packed-blob column layout (fp32 columns of a [128, BLOBW] tile)
OFF_WIH = 0            # 4*192: [k, gate-col] of w_ih slice .T
OFF_WHH = 768
OFF_XH = 1536          # partitions 0..9: [x | h0] (2*512)
OFF_H0S = 2560         # partitions 0..9: h0 gate-row slice (64)
OFF_BIH = 2624         # partition 0: b_ih slice (192)
OFF_BHH = 2816
OFF_ONES = 3008        # partition 0: ten 1.0s
OFF_BCAT = 3018        # all partitions: b_concat chunks (4)
OFF_IDX = 3022         # all partitions: 120 int16 gather idx = 60 fp32 cols
BLOBW = 3082

_CACHE = {}


def _build_program():
    import concourse.bass as bass
    import concourse.tile as tile
    from concourse import bacc, mybir
    from concourse.masks import make_identity

    fp32 = mybir.dt.float32
    i16 = mybir.dt.int16
    AF = mybir.ActivationFunctionType

    nc = bacc.Bacc("TRN2", target_bir_lowering=False, debug=False,
                   enable_asserts=False, num_devices=NCORES)

    def din(name, shape, dt=fp32):
        return nc.dram_tensor(name, list(shape), dt, kind="ExternalInput").ap()

    def dout(name, shape, dt=fp32):
        return nc.dram_tensor(name, list(shape), dt, kind="ExternalOutput").ap()

    blob = din("blob", (128, BLOBW))
    encs = din("encs", (LS, B, H))
    wcat = din("wcat", (2 * H, H))       # w_concat.T
    wvo = din("wvo", (H, VS))            # w_out slab .T
    bvo = din("bvo", (1, VS))
    embkb = din("embkb", (VOCAB, H))

    logits = dout("logits", (B, VS))
    attnw = dout("attnw", (LS, B))
    h1T_o = dout("h1T", (H, B))
    ctx_o = dout("ctxo", (B, H))
    e2c = dout("e2c", (B * NJ, H))

    h1cc = nc.dram_tensor("h1cc", [GS, B], fp32).ap()
    h1all = nc.dram_tensor("h1all", [H, B], fp32, addr_space="Shared").ap()
    ctxcc = nc.dram_tensor("ctxcc", [B, H], fp32).ap()
    ctxall = nc.dram_tensor("ctxall", [B, H], fp32, addr_space="Shared").ap()

    groups = [list(range(NCORES))]

    with tile.TileContext(nc) as tc, \
         tc.tile_pool(name="const", bufs=1) as constp, \
         tc.tile_pool(name="sb", bufs=1) as sb, \
         tc.tile_pool(name="psT", bufs=2, space="PSUM") as psT, \
         tc.tile_pool(name="psM", bufs=4, space="PSUM") as psM:

        ident = constp.tile([128, 128], fp32)
        make_identity(nc, ident[:])

        # ---------- input DMAs ----------
        bl = sb.tile([128, BLOBW], fp32, tag="blob")
        nc.sync.dma_start(bl[:], blob[:])
        wvo_s = sb.tile([128, 4, VS], fp32, tag="wvo")
        nc.sync.dma_start(wvo_s[:], wvo.rearrange("(k p) n -> p k n", p=128))

        E = sb.tile([LS, B * H], fp32, tag="E")
        nc.scalar.dma_start(E[:], encs.rearrange("l b h -> l (b h)"))
        wcat_s = sb.tile([128, 8, H], fp32, tag="wcat")
        nc.scalar.dma_start(wcat_s[:], wcat.rearrange("(k p) m -> p k m", p=128))
        bvo_s = sb.tile([1, VS], fp32, tag="bvo")
        nc.scalar.dma_start(bvo_s[:], bvo[:])

        ones10 = bl[0:1, OFF_ONES:OFF_ONES + B]
        idx16 = bl[:, OFF_IDX:OFF_IDX + 60].bitcast(i16)

        # kb gather, split so collective triggers interleave on the Q7 queue
        kbg1 = sb.tile([128, 5, H], fp32, tag="kbg1")
        nc.gpsimd.dma_gather(kbg1[:], embkb[:], idx16[:, 0:NI1 // 16],
                             NI1, NI1, H, single_packet=False)

        # ---------- transposes of x / h0 ----------
        xT = sb.tile([128, 4, B], fp32, tag="xT")
        h0T = sb.tile([128, 4, B], fp32, tag="h0T")
        for t, dst in ((0, xT), (1, h0T)):
            for j in range(4):
                ps = psT.tile([128, B], fp32, tag="t")
                o = OFF_XH + t * 512 + j * 128
                nc.tensor.transpose(ps[:], bl[0:B, o:o + 128], ident[:B, :B])
                nc.vector.tensor_copy(dst[:, j, :], ps[:])
        ps_h0s = psT.tile([GS, B], fp32, tag="t")
        nc.tensor.transpose(ps_h0s[:], bl[0:B, OFF_H0S:OFF_H0S + GS],
                            ident[:B, :B])
        h0sT = sb.tile([GS, B], fp32, tag="h0sT")
        nc.vector.tensor_copy(h0sT[:], ps_h0s[:])

        brz = sb.tile([1, 2 * GS], fp32, tag="brz")
        nc.vector.tensor_add(brz[:], bl[0:1, OFF_BIH:OFF_BIH + 2 * GS],
                             bl[0:1, OFF_BHH:OFF_BHH + 2 * GS])

        # ---------- GRU (sharded gates, partitions 0..63) ----------
        def wih_sl(k, g):
            o = OFF_WIH + k * 192 + g * GS
            return bl[:, o:o + GS]

        def whh_sl(k, g):
            o = OFF_WHH + k * 192 + g * GS
            return bl[:, o:o + GS]

        ps_r = psM.tile([GS, B], fp32, tag="m")
        ps_z = psM.tile([GS, B], fp32, tag="m")
        ps_gin = psM.tile([GS, B], fp32, tag="m")
        ps_ghn = psM.tile([GS, B], fp32, tag="m")
        for g, ps in ((0, ps_r), (1, ps_z)):
            nc.tensor.matmul(ps[:], brz[:, g * GS:(g + 1) * GS], ones10,
                             start=True, stop=False)
            for k in range(4):
                nc.tensor.matmul(ps[:], wih_sl(k, g), xT[:, k, :],
                                 start=False, stop=False)
            for k in range(4):
                nc.tensor.matmul(ps[:], whh_sl(k, g), h0T[:, k, :],
                                 start=False, stop=(k == 3))
        for wsl, boff, ps, src in ((wih_sl, OFF_BIH, ps_gin, xT),
                                   (whh_sl, OFF_BHH, ps_ghn, h0T)):
            nc.tensor.matmul(ps[:], bl[0:1, boff + 2 * GS:boff + 3 * GS], ones10,
                             start=True, stop=False)
            for k in range(4):
                nc.tensor.matmul(ps[:], wsl(k, 2), src[:, k, :],
                                 start=False, stop=(k == 3))

        r_sb = sb.tile([GS, B], fp32, tag="r")
        z_sb = sb.tile([GS, B], fp32, tag="z")
        nc.scalar.activation(r_sb[:], ps_r[:], AF.Sigmoid)
        nc.scalar.activation(z_sb[:], ps_z[:], AF.Sigmoid)
        tmpn = sb.tile([GS, B], fp32, tag="tmpn")
        nc.vector.tensor_mul(tmpn[:], r_sb[:], ps_ghn[:])
        nc.vector.tensor_add(tmpn[:], tmpn[:], ps_gin[:])
        n_sb = sb.tile([GS, B], fp32, tag="n")
        nc.scalar.activation(n_sb[:], tmpn[:], AF.Tanh)
        zn = sb.tile([GS, B], fp32, tag="zn")
        nc.vector.tensor_mul(zn[:], z_sb[:], n_sb[:])
        zh = sb.tile([GS, B], fp32, tag="zh")
        nc.vector.tensor_mul(zh[:], z_sb[:], h0sT[:])
        h1c = sb.tile([GS, B], fp32, tag="h1c")
        nc.vector.tensor_sub(h1c[:], n_sb[:], zn[:])
        nc.vector.tensor_add(h1c[:], h1c[:], zh[:])

        # AllGather h1 (trigger on gpsimd queue, between the two gather gens)
        nc.scalar.dma_start(h1cc[:], h1c[:])
        nc.gpsimd.collective_compute("AllGather", mybir.AluOpType.bypass,
                                     replica_groups=groups,
                                     ins=[h1cc[:]], outs=[h1all[:]])
        h1Ts = sb.tile([128, 4, B], fp32, tag="h1Ts")
        nc.scalar.dma_start(h1Ts[:], h1all.rearrange("(j p) b -> p j b", p=128))
        nc.scalar.dma_start(h1T_o[:], h1all[:])

        # second kb gather gen runs while the AllGather is in flight
        kbg2 = sb.tile([128, 10, H], fp32, tag="kbg2")
        nc.gpsimd.dma_gather(kbg2[:], embkb[:], idx16[:, NI1 // 16:NIDX // 16],
                             NI2, NI2, H, single_packet=False)

        # ---------- attention ----------
        ET = sb.tile([128, 40, LS], fp32, tag="ET")
        for ci in range(40):
            ps = psT.tile([128, LS], fp32, tag="t")
            nc.tensor.transpose(ps[:], E[:, ci * 128:(ci + 1) * 128],
                                ident[:LS, :LS])
            nc.vector.tensor_copy(ET[:, ci, :], ps[:])

        ps_en = psM.tile([LS, B], fp32, tag="m")
        for b in range(B):
            for j in range(4):
                nc.tensor.matmul(ps_en[:, b:b + 1], ET[:, b * 4 + j, :],
                                 h1Ts[:, j, b:b + 1],
                                 start=(j == 0), stop=(j == 3))
        en_lb = sb.tile([LS, B], fp32, tag="en_lb")
        nc.vector.tensor_copy(en_lb[:], ps_en[:])

        mx = sb.tile([LS, 1], fp32, tag="mx")
        nc.vector.reduce_max(out=mx[:], in_=en_lb[:], axis=mybir.AxisListType.X)
        nmx = sb.tile([LS, 1], fp32, tag="nmx")
        nc.vector.tensor_scalar_mul(nmx[:], mx[:], -1.0)
        ex = sb.tile([LS, B], fp32, tag="ex")
        nc.scalar.activation(ex[:], en_lb[:], AF.Exp, bias=nmx[:])
        sm = sb.tile([LS, 1], fp32, tag="sm")
        nc.vector.reduce_sum(out=sm[:], in_=ex[:], axis=mybir.AxisListType.X)
        rs = sb.tile([LS, 1], fp32, tag="rs")
        nc.vector.reciprocal(rs[:], sm[:])
        attn = sb.tile([LS, B], fp32, tag="attn")
        nc.vector.tensor_scalar_mul(attn[:], ex[:], rs[:])
        nc.scalar.dma_start(attnw[:], attn[:])

        adiag = sb.tile([LS, B * B], fp32, tag="adiag")
        nc.vector.memset(adiag[:], 0.0)
        for b in range(B):
            nc.vector.tensor_copy(adiag[:, b * B + b:b * B + b + 1],
                                  attn[:, b:b + 1])
        ps_ctx = psM.tile([B, H], fp32, tag="m")
        for b in range(B):
            nc.tensor.matmul(ps_ctx[:], adiag[:, b * B:(b + 1) * B],
                             E[:, b * H:(b + 1) * H],
                             start=(b == 0), stop=(b == B - 1))
        ctx_sb = sb.tile([B, H], fp32, tag="ctx_sb")
        nc.vector.tensor_copy(ctx_sb[:], ps_ctx[:])
        nc.scalar.dma_start(ctxcc[:], ctx_sb[:])
        nc.gpsimd.collective_compute("AllReduce", mybir.AluOpType.add,
                                     replica_groups=groups,
                                     ins=[ctxcc[:]], outs=[ctxall[:]])
        ctxf = sb.tile([B, H], fp32, tag="ctxf")
        nc.scalar.dma_start(ctxf[:], ctxall[:])
        nc.scalar.dma_start(ctx_o[:], ctxall[:])

        ctxT = sb.tile([128, 4, B], fp32, tag="ctxT")
        for j in range(4):
            ps = psT.tile([128, B], fp32, tag="t")
            nc.tensor.transpose(ps[:], ctxf[:, j * 128:(j + 1) * 128],
                                ident[:B, :B])
            nc.vector.tensor_copy(ctxT[:, j, :], ps[:])

        # ---------- concat projection (tanh) ----------
        coT = sb.tile([128, 4, B], fp32, tag="coT")
        for m in range(4):
            ps = psM.tile([128, B], fp32, tag="m")
            for k in range(8):
                rhs = h1Ts[:, k, :] if k < 4 else ctxT[:, k - 4, :]
                nc.tensor.matmul(ps[:], wcat_s[:, k, m * 128:(m + 1) * 128], rhs,
                                 start=(k == 0), stop=(k == 7))
            nc.scalar.activation(coT[:, m, :], ps[:], AF.Tanh,
                                 bias=bl[:, OFF_BCAT + m:OFF_BCAT + m + 1])

        # ---------- vocab projection ----------
        for ns in range(8):
            n0 = ns * 512
            nn = min(VS, n0 + 512) - n0
            ps = psM.tile([B, 512], fp32, tag="m")
            nc.tensor.matmul(ps[:, :nn], ones10, bvo_s[:, n0:n0 + nn],
                             start=True, stop=False)
            for k in range(4):
                nc.tensor.matmul(ps[:, :nn], coT[:, k, :], wvo_s[:, k, n0:n0 + nn],
                                 start=False, stop=(k == 3))
            lg = sb.tile([B, 512], fp32, tag=f"lg{ns % 2}")
            nc.vector.tensor_copy(lg[:, :nn], ps[:, :nn])
            nc.sync.dma_start(logits[:, n0:n0 + nn], lg[:, :nn])

        # ---------- kb sum + writeback ----------
        e2sum = sb.tile([128, 5, H], fp32, tag="e2sum")
        nc.vector.tensor_add(e2sum[:], kbg1[:], kbg2[:, 0:5, :])
        nc.vector.tensor_add(e2sum[:], e2sum[:], kbg2[:, 5:10, :])
        nc.sync.dma_start(e2c.rearrange("(cc p) h -> p cc h", p=128), e2sum[:])

    nc.compile()
    return nc


def _get_program():
    if "nc" not in _CACHE:
        _CACHE["nc"] = _build_program()
    return _CACHE["nc"]


def _j_range(c):
    lo = (c * SLAB) // H
    hi = -((-(c + 1) * SLAB) // H)  # ceil
    return lo, hi


def _prep_inputs(input_seq, kb_inputs, last_context, last_hidden, encoder_outputs,
                 emb, emb_kb, w_ih, w_hh, b_ih, b_hh, w_concat, b_concat,
                 w_out, b_out):
    f = np.float32
    x = np.ascontiguousarray(emb[np.asarray(input_seq).astype(np.int64)], dtype=f)
    h0 = np.ascontiguousarray(last_hidden[0], dtype=f)
    wcat = np.ascontiguousarray(np.asarray(w_concat, dtype=f).T)
    embkb = np.ascontiguousarray(emb_kb, dtype=f)
    enc = np.asarray(encoder_outputs, dtype=f)
    kbi = np.asarray(kb_inputs).astype(np.int64)
    w_ih = np.asarray(w_ih, f)
    w_hh = np.asarray(w_hh, f)
    b_ih = np.asarray(b_ih, f)
    b_hh = np.asarray(b_hh, f)
    bcat_chunks = np.asarray(b_concat, f).reshape(4, 128).T  # (128, 4)

    in_maps = []
    for c in range(NCORES):
        rows = np.r_[c * GS:(c + 1) * GS, H + c * GS:H + (c + 1) * GS,
                     2 * H + c * GS:2 * H + (c + 1) * GS]
        lo, hi = _j_range(c)
        jc = hi - lo
        idx = np.zeros((3, B, NJ), np.int16)
        idx[:, :, :jc] = kbi[:, lo:hi, :].transpose(2, 0, 1)
        idx_w = np.tile(idx.reshape(NIDX // 16, 16).T, (8, 1))  # (128, 120)

        blob = np.zeros((128, BLOBW), f)
        blob[:, OFF_WIH:OFF_WIH + 768] = (
            w_ih[rows].T.reshape(4, 128, 192).transpose(1, 0, 2).reshape(128, 768))
        blob[:, OFF_WHH:OFF_WHH + 768] = (
            w_hh[rows].T.reshape(4, 128, 192).transpose(1, 0, 2).reshape(128, 768))
        blob[0:B, OFF_XH:OFF_XH + 512] = x
        blob[0:B, OFF_XH + 512:OFF_XH + 1024] = h0
        blob[0:B, OFF_H0S:OFF_H0S + GS] = h0[:, c * GS:(c + 1) * GS]
        blob[0, OFF_BIH:OFF_BIH + 192] = b_ih[rows]
        blob[0, OFF_BHH:OFF_BHH + 192] = b_hh[rows]
        blob[0, OFF_ONES:OFF_ONES + B] = 1.0
        blob[:, OFF_BCAT:OFF_BCAT + 4] = bcat_chunks
        blob[:, OFF_IDX:OFF_IDX + 60] = np.ascontiguousarray(idx_w).view(f)

        m = {
            "blob": blob,
            "encs": np.ascontiguousarray(enc[c * LS:(c + 1) * LS]),
            "wcat": wcat,
            "wvo": np.ascontiguousarray(np.asarray(w_out, f)[c * VS:(c + 1) * VS].T),
            "bvo": np.ascontiguousarray(np.asarray(b_out, f)[c * VS:(c + 1) * VS][None]),
            "embkb": embkb,
        }
        in_maps.append(m)
    return in_maps


def _assemble(results):
    f = np.float32
    out = np.concatenate([results[c]["logits"] for c in range(NCORES)], axis=1)
    h1 = np.ascontiguousarray(results[0]["h1T"].T)[None]          # (1, B, H)
    context = results[0]["ctxo"]
    attn_full = np.concatenate([results[c]["attnw"] for c in range(NCORES)], axis=0)
    attn_weights = np.ascontiguousarray(attn_full.T)[:, None, :]   # (B, 1, L)
    kb_attn = np.zeros((B, H, KB_PAD + KB), f)
    for c in range(NCORES):
        lo, hi = _j_range(c)
        jc = hi - lo
        off = c * SLAB - lo * H
        e2 = results[c]["e2c"].reshape(B, NJ, H)
        for b in range(B):
            seg = e2[b, :jc].reshape(-1)[off:off + SLAB]
            kb_attn[b, c * LS:(c + 1) * LS, KB_PAD:] = seg.reshape(LS, KB)
    return (np.ascontiguousarray(out), np.ascontiguousarray(context),
            h1, attn_weights, kb_attn)


def run_sim(**inputs):
    """Run via the multi-core interpreter (correctness check, no HW)."""
    from concourse.bass_interp import MultiCoreSim
    nc = _get_program()
    in_maps = _prep_inputs(**inputs)
    sim = MultiCoreSim(nc, num_cores=NCORES, trace=False)
    for c in range(NCORES):
        for k, v in in_maps[c].items():
            sim.cores[c].tensor(k)[:] = v
    sim.simulate()
    results = [{k: np.array(sim.cores[c].tensor(k))
                for k in ("logits", "attnw", "h1T", "ctxo", "e2c")}
               for c in range(NCORES)]
    return _assemble(results)


def kernel(**inputs):
    from concourse.bass_utils import run_bass_kernel_spmd
    nc = _get_program()
    in_maps = _prep_inputs(**inputs)
    res = run_bass_kernel_spmd(nc, in_maps, list(range(NCORES)))
    return _assemble(res.results)


def kernel_profiled(**inputs):
    from concourse.bass_utils import run_bass_kernel_spmd
    nc = _get_program()
    in_maps = _prep_inputs(**inputs)
    res = run_bass_kernel_spmd(nc, in_maps, list(range(NCORES)), trace=True)
    return _assemble(res.results), res
